# revision 28
# baseline (speedup 1.0000x reference)
"""Trainium2 Bass kernel for nn_Euler: 512-step Euler integration of a
2-layer tanh MLP, data-parallel over 8 NeuronCores (batch 1024 -> 128/core).

Math per core (transposed internal orientation, state on S partitions):
  zT = [stateT; uT; ones] (97 partitions x 128 batch), split bf16 hi/lo.
  mm1 (bf16 hi/lo 3-term): psum_h[128, 4*128] = chunks of (z @ [W1;b1]).T
  tanh: ACT psum -> h fp32 SBUF
  mm2 (fp32): diffT = (DT*W2).T @ h chunks + DT*b2, accumulated in PSUM
  update: DVE stateT += diffT; re-split state to bf16 hi/lo for next step.
State is carried in fp32 end-to-end; matmul precision ~2e-4 rel vs fp32.

Perf-critical host-side design (the benchmark measures warm end-to-end
kernel() wall-clock, which here is dominated by axon-tunnel transfers at
~100MB/s, not device compute — the 512-step NEFF itself runs in a few ms):
  - All I/O uses the NATURAL layout of the reference inputs/outputs, so the
    host does no transposes of the big tensors:
      u      per-core [128, L, U] fp16  (one astype pass; halves the upload,
                                         adds ~8e-4 rel err vs the 2e-2 gate)
      s0     per-core [128, S]    fp32  == initial_state row block (zero-copy)
      out    per-core [128, L, S] bf16  -> global (B, L, S), one astype to f32
                                         (halves the download, ~3e-3 rel err)
    The per-step transpose of u (128xU -> Ux128) and of the output state
    (Sx128 -> 128xS), plus the bf16 hi/lo splits, are done ON DEVICE
    (PE transpose via an on-device identity + DVE copies).
  - The shard_map executor (same mechanism run_bass_kernel_spmd uses under
    axon, see concourse.bass2jax.run_bass_via_pjrt) is built ONCE and cached,
    so warm calls skip re-trace/re-lower/re-compile.
  - Output operands are donated from the previous call's device buffers
    (every element is rewritten) instead of transferring host zeros each call.
"""

import numpy as np
from contextlib import ExitStack

B, L, S, U, H = 1024, 512, 64, 32, 512
DT = 0.1
NCORES = 8
BLOC = B // NCORES  # 128
KZ = S + U + 1      # 97 (state + control + bias row)
NCH = H // 128      # 4 H-chunks

_RUNNER = None


def _build(nsteps):
    import concourse.bass as cbass
    import concourse.bacc as bacc
    import concourse.tile as tile
    import concourse.mybir as mybir

    F32 = mybir.dt.float32
    F16 = mybir.dt.bfloat16  # hi/lo split dtype: bf16 avoids fp16-subnormal slow path
    FP16 = mybir.dt.float16  # u upload dtype: halves host->device transfer
    TANH = mybir.ActivationFunctionType.Tanh
    COPY = mybir.ActivationFunctionType.Copy
    ADD = mybir.AluOpType.add
    SUB = mybir.AluOpType.subtract
    ds = cbass.ds

    nc = bacc.Bacc("TRN2", target_bir_lowering=False, debug=False,
                   num_devices=NCORES)

    s0_d = nc.dram_tensor("s0", [BLOC, S], F32, kind="ExternalInput").ap()
    u_d = nc.dram_tensor("u", [BLOC, nsteps, U], FP16, kind="ExternalInput").ap()
    w1hi_d = nc.dram_tensor("w1hi", [KZ, H], F16, kind="ExternalInput").ap()
    w1lo_d = nc.dram_tensor("w1lo", [KZ, H], F16, kind="ExternalInput").ap()
    w2_d = nc.dram_tensor("w2", [NCH, 128, S], F32, kind="ExternalInput").ap()
    b2_d = nc.dram_tensor("b2row", [1, S], F32, kind="ExternalInput").ap()
    # bf16 output halves the device->host transfer; states are O(1)-O(100)
    # magnitude so bf16 quantization (~2^-9 rel) is far inside the 2e-2 gate
    out_d = nc.dram_tensor("out", [BLOC, nsteps, S], F16, kind="ExternalOutput").ap()
    # exact f32 final state, for chaining chunked invocations device-side
    sfin_d = nc.dram_tensor("sfin", [BLOC, S], F32, kind="ExternalOutput").ap()

    UNROLL = 16
    assert nsteps % UNROLL == 0 and nsteps >= 2 * UNROLL

    with tile.TileContext(nc) as tc, ExitStack() as ctx:
        cpool = ctx.enter_context(tc.tile_pool(name="const", bufs=1))
        spool = ctx.enter_context(tc.tile_pool(name="state", bufs=1))
        hpool = ctx.enter_context(tc.tile_pool(name="h", bufs=2))
        opool = ctx.enter_context(tc.tile_pool(name="outs", bufs=3))
        pp_h = ctx.enter_context(tc.tile_pool(name="ps_h", bufs=2, space="PSUM"))
        pp_d = ctx.enter_context(tc.tile_pool(name="ps_d", bufs=2, space="PSUM"))
        pp_u = ctx.enter_context(tc.tile_pool(name="ps_u", bufs=2, space="PSUM"))
        pp_o = ctx.enter_context(tc.tile_pool(name="ps_o", bufs=2, space="PSUM"))

        # --- static weights/constants ---
        w1hi = cpool.tile([KZ, H], F16)
        w1lo = cpool.tile([KZ, H], F16)
        w2 = cpool.tile([128, NCH * S], F32)
        b2r = cpool.tile([1, S], F32)
        ones = cpool.tile([1, BLOC], F32)
        ident = cpool.tile([128, 128], F32)
        nc.sync.dma_start(w1hi[:, :], w1hi_d[:, :])
        nc.sync.dma_start(w1lo[:, :], w1lo_d[:, :])
        for j in range(NCH):
            nc.sync.dma_start(w2[:, j * S:(j + 1) * S], w2_d[j, :, :])
        nc.sync.dma_start(b2r[:, :], b2_d[:, :])
        from concourse.masks import make_identity
        make_identity(nc, ident[:, :])
        nc.vector.memset(ones[:, :], 1.0)

        # --- double-buffered z (hi/lo), state, and u-staging tiles ---
        zhi = [spool.tile([KZ, BLOC], F16, tag=f"zhi{i}", name=f"zhi{i}") for i in range(2)]
        zlo = [spool.tile([KZ, BLOC], F16, tag=f"zlo{i}", name=f"zlo{i}") for i in range(2)]
        sT = [spool.tile([S, BLOC], F32, tag=f"sT{i}", name=f"sT{i}") for i in range(2)]
        us = [spool.tile([BLOC, U], FP16, tag=f"us{i}", name=f"us{i}") for i in range(2)]
        us32 = [spool.tile([BLOC, U], F32, tag=f"us32_{i}", name=f"us32_{i}") for i in range(2)]
        for i in range(2):
            nc.vector.memset(zhi[i][S + U:KZ, :], 1.0)   # bias row (hi = 1.0)
            nc.vector.memset(zlo[i][S + U:KZ, :], 0.0)   # bias row (lo = 0)

        # --- prologue: seed state + u_0 from natural-layout DRAM ---
        stile = spool.tile([BLOC, S], F32, tag="stile", name="stile")
        nc.sync.dma_start(stile[:, :], s0_d[:, :])
        nc.sync.dma_start(us[0][:, :], u_d[:, 0, :])
        # borrow a ph-pool PSUM tile for the one-time s0 transpose (PSUM is
        # bank-granular: a dedicated tile would not fit in the 8 banks)
        pt = pp_h.tile([128, H], F32, tag="ph", name="ph_pro")
        nc.tensor.transpose(pt[:S, :BLOC], stile[:, :], ident[:, :])
        nc.vector.tensor_copy(sT[0][:, :], pt[:S, :BLOC])
        nc.vector.tensor_copy(zhi[0][:S, :], sT[0][:, :])
        nc.vector.tensor_tensor(zlo[0][:S, :], sT[0][:, :], zhi[0][:S, :], SUB)
        pu0 = pp_u.tile([U, BLOC], F32, tag="pu", name="pu_pro")
        nc.vector.tensor_copy(us32[0][:, :], us[0][:, :])
        nc.tensor.transpose(pu0[:, :], us32[0][:, :], ident[:, :])
        nc.vector.tensor_copy(zhi[0][S:S + U, :], pu0[:, :])
        nc.vector.tensor_tensor(zlo[0][S:S + U, :], pu0[:, :], zhi[0][S:S + U, :], SUB)

        def step_body(t, k, prefetch=True):
            """One Euler step. t may be a loop register expr or a python int;
            k is the unrolled offset (parity must match t%2). When prefetch is
            False (last step only) skip loading/preparing u_{t+1}."""
            X = k % 2
            Y = (k + 1) % 2
            # prefetch next-step control into the staging tile (natural layout)
            if prefetch:
                nc.sync.dma_start(us[Y][:, :], u_d[:, ds(t + 1, 1), :])
            # mm1: 12 bf16 matmuls -> psum_h (hT chunks)
            ph = pp_h.tile([128, H], F32, tag="ph", name=f"ph{k}")
            for j in range(NCH):
                o = ph[:, j * 128:(j + 1) * 128]
                wj = slice(j * 128, (j + 1) * 128)
                nc.tensor.matmul(o, w1hi[:, wj], zhi[X][:, :], start=True, stop=False)
                nc.tensor.matmul(o, w1hi[:, wj], zlo[X][:, :], start=False, stop=False)
                nc.tensor.matmul(o, w1lo[:, wj], zhi[X][:, :], start=False, stop=True)
            # tanh split in two ACT instructions so mm2 chunks 0-1 start early
            nsp = 2
            h = hpool.tile([128, H], F32, tag="h", name=f"h{k}")
            cw = H // nsp
            for p in range(nsp):
                nc.scalar.activation(h[:, p * cw:(p + 1) * cw],
                                     ph[:, p * cw:(p + 1) * cw], TANH)
            # transpose u_{t+1} (natural 128xU -> Ux128) and split bf16 hi/lo
            if prefetch:
                pu = pp_u.tile([U, BLOC], F32, tag="pu", name=f"pu{k}")
                nc.vector.tensor_copy(us32[Y][:, :], us[Y][:, :])
                nc.tensor.transpose(pu[:, :], us32[Y][:, :], ident[:, :])
                nc.vector.tensor_copy(zhi[Y][S:S + U, :], pu[:, :])
                nc.vector.tensor_tensor(zlo[Y][S:S + U, :], pu[:, :],
                                        zhi[Y][S:S + U, :], SUB)
            # mm2: fp32, accumulate 4 chunks + bias row
            pd = pp_d.tile([128, BLOC], F32, tag="pd", name=f"pd{k}")
            nc.tensor.matmul(pd[:S, :], b2r[:, :], ones[:, :], start=True, stop=False)
            for j in range(NCH):
                nc.tensor.matmul(
                    pd[:S, :], w2[:, j * S:(j + 1) * S],
                    h[:, j * 128:(j + 1) * 128],
                    start=False, stop=(j == NCH - 1),
                )
            # state update + re-split (fp32 carried state)
            nc.vector.tensor_tensor(sT[Y][:, :], sT[X][:, :], pd[:S, :], ADD)
            nc.vector.tensor_copy(zhi[Y][:S, :], sT[Y][:, :])
            nc.vector.tensor_tensor(zlo[Y][:S, :], sT[Y][:, :], zhi[Y][:S, :], SUB)
            # output: transpose to natural (128xS), stage in SBUF, DMA out row t
            po = pp_o.tile([BLOC, S], F32, tag="po", name=f"po{k}")
            nc.tensor.transpose(po[:, :], sT[Y][:, :], ident[:S, :S])
            ot = opool.tile([BLOC, S], F16, tag="ot", name=f"ot{k}")
            nc.scalar.activation(ot[:, :], po[:, :], COPY)
            nc.sync.dma_start(out_d[:, ds(t, 1), :], ot[:, :])

        with tc.For_i(0, nsteps - UNROLL, UNROLL,
                      hint_engines=(mybir.EngineType.PE,)) as iv:
            for k in range(UNROLL):
                step_body(iv + k, k)
        # peeled static tail: last UNROLL steps; final step skips u prefetch
        for t in range(nsteps - UNROLL, nsteps):
            step_body(t, t - (nsteps - UNROLL), prefetch=(t < nsteps - 1))

        # epilogue: write the exact f32 final state (natural layout)
        pf = pp_o.tile([BLOC, S], F32, tag="po", name="po_fin")
        nc.tensor.transpose(pf[:, :], sT[nsteps % 2][:, :], ident[:S, :S])
        sf = opool.tile([BLOC, S], F32, tag="sf", name="sf_fin")
        nc.scalar.activation(sf[:, :], pf[:, :], COPY)
        nc.sync.dma_start(sfin_d[:, :], sf[:, :])

    nc.compile()
    return nc


class _Runner:
    """Cached shard_map executor for the compiled Bass program.

    Mirrors concourse.bass2jax.run_bass_via_pjrt (the path
    bass_utils.run_bass_kernel_spmd takes under axon) but builds the jitted
    callable once, and sources the donated output buffers from device-side
    jnp.zeros rather than host zeros.
    """

    def __init__(self, chunk):
        import jax
        import jax.numpy as jnp
        from jax.sharding import Mesh, PartitionSpec, NamedSharding
        from jax.experimental.shard_map import shard_map
        import concourse.mybir as mybir
        from concourse import bass2jax

        self.chunk = chunk
        nc = _build(chunk)
        self.nc = nc
        bass2jax.install_neuronx_cc_hook()

        partition_name = (nc.partition_id_tensor.name
                          if nc.partition_id_tensor is not None else None)
        assert nc.dbg_addr is None, "built with debug=False"

        in_names, out_names, out_avals = [], [], []
        for alloc in nc.m.functions[0].allocations:
            if not isinstance(alloc, mybir.MemoryLocationSet):
                continue
            assert alloc.memorylocations
            name = alloc.memorylocations[0].name
            if alloc.kind == "ExternalInput":
                if name != partition_name:
                    in_names.append(name)
            elif alloc.kind == "ExternalOutput":
                assert alloc.tensor_shape is not None and alloc.dtype is not None
                out_names.append(name)
                out_avals.append(jax.core.ShapedArray(
                    tuple(alloc.tensor_shape), mybir.dt.np(alloc.dtype)))
        assert sorted(out_names) == ["out", "sfin"], out_names
        self.i_out = out_names.index("out")
        self.i_sfin = out_names.index("sfin")
        self.in_names = list(in_names)
        n_params = len(in_names)
        n_outs = len(out_names)
        all_in_names = tuple(in_names + out_names
                             + ([partition_name] if partition_name else []))
        out_avals_t = tuple(out_avals)
        out_names_t = tuple(out_names)

        def _body(*args):
            operands = list(args)
            if partition_name is not None:
                operands.append(bass2jax.partition_id_tensor())
            outs = bass2jax._bass_exec_p.bind(
                *operands,
                out_avals=out_avals_t,
                in_names=all_in_names,
                out_names=out_names_t,
                lowering_input_output_aliases=(),
                sim_require_finite=True,
                sim_require_nnan=True,
                nc=nc,
            )
            return tuple(outs)

        devices = jax.devices()[:NCORES]
        assert len(devices) == NCORES
        self.devices = devices
        mesh = Mesh(np.asarray(devices), ("core",))
        self.ushard = NamedSharding(mesh, PartitionSpec("core"))
        donate = tuple(range(n_params, n_params + n_outs))
        self.sharded = jax.jit(
            shard_map(_body, mesh=mesh,
                      in_specs=(PartitionSpec("core"),) * (n_params + n_outs),
                      out_specs=(PartitionSpec("core"),) * n_outs,
                      check_rep=False),
            donate_argnums=donate, keep_unused=True)
        zshard = NamedSharding(mesh, PartitionSpec("core"))
        gshapes = [(NCORES * a.shape[0], *a.shape[1:]) for a in out_avals]
        gdtypes = [a.dtype for a in out_avals]
        self.zero_fn = jax.jit(
            lambda: tuple(jnp.zeros(s, d) for s, d in zip(gshapes, gdtypes)),
            out_shardings=(zshard,) * n_outs)
        # donated output operands for the next call: the kernel writes every
        # output element, so after materializing a result to host we can hand
        # its device buffers straight back as donors (content irrelevant)
        self._donors = None

    def put_u(self, u_f32):
        """Upload one chunk of control inputs: convert each core's shard to
        fp16 and device_put it immediately, so shard c+1's host-side convert
        overlaps shard c's (async) transfer on the tunnel."""
        import jax
        pieces = [
            jax.device_put(u_f32[c * BLOC:(c + 1) * BLOC].astype(np.float16),
                           self.devices[c])
            for c in range(NCORES)
        ]
        return jax.make_array_from_single_device_arrays(
            (B, u_f32.shape[1], U), self.ushard, pieces)

    def run(self, s0, u_chunks, weights):
        """Run len(u_chunks) sequential device invocations, chaining the f32
        state on device, and pipeline host<->device transfers: chunk c+1's
        upload+exec overlaps chunk c's output download."""
        nchunks = len(u_chunks)
        if self._donors is None or len(self._donors) != nchunks:
            self._donors = [list(self.zero_fn()) for _ in range(nchunks)]
        outs = []
        s = s0
        for c in range(nchunks):
            arrays = {**weights, "s0": s, "u": u_chunks[c]}
            args = [arrays[n] for n in self.in_names]
            res = self.sharded(*args, *self._donors[c])
            outs.append(res)
            s = res[self.i_sfin]
        chunk = self.chunk
        full = np.empty((B, chunk * nchunks, S), np.float32)
        for c in range(nchunks):
            o = outs[c][self.i_out]
            dst = full[:, c * chunk:(c + 1) * chunk]
            # start all shard downloads, then convert each shard bf16->f32
            # while the later shards are still in flight on the tunnel
            shards = o.addressable_shards
            for sh in shards:
                sh.data.copy_to_host_async()
            for sh in shards:
                dst[sh.index] = np.asarray(sh.data)
        self._donors = [list(res) for res in outs]
        return full


def _prep_weights(W1, b1, W2, b2):
    import ml_dtypes
    f32 = np.float32
    bf16 = ml_dtypes.bfloat16
    W1b = np.concatenate([np.asarray(W1, f32), np.asarray(b1, f32)[None, :]],
                         axis=0)                       # (97, 512)
    w1hi = W1b.astype(bf16)
    w1lo = (W1b - w1hi.astype(f32)).astype(bf16)
    w2 = (np.asarray(W2, f32) * f32(DT)).reshape(NCH, 128, S)
    b2r = (np.asarray(b2, f32) * f32(DT))[None, :]

    def tile8(a):
        return np.tile(a, (NCORES,) + (1,) * (a.ndim - 1))

    return {
        "w1hi": tile8(w1hi), "w1lo": tile8(w1lo),
        "w2": tile8(w2), "b2row": tile8(b2r),
    }


import os as _os
# chunks per call: >1 overlaps chunk c's download with chunk c+1's work, but
# measured per-dispatch overhead through axon outweighs the overlap; keep 1
NSPLIT = int(_os.environ.get("KNSPLIT", "1"))


def kernel(initial_state, control_inputs, W1, b1, W2, b2, nsteps=L):
    global _RUNNER
    nspl = NSPLIT if (nsteps % NSPLIT == 0
                      and (nsteps // NSPLIT) % 16 == 0
                      and nsteps // NSPLIT >= 32) else 1
    chunk = nsteps // nspl
    if _RUNNER is None or _RUNNER.chunk != chunk:
        _RUNNER = _Runner(chunk)
    ci = np.asarray(control_inputs)
    # start the u upload first: weight prep below overlaps the transfer
    u_chunks = [_RUNNER.put_u(ci[:, c * chunk:(c + 1) * chunk])
                for c in range(nspl)]
    weights = _prep_weights(W1, b1, W2, b2)
    s0 = np.ascontiguousarray(np.asarray(initial_state, np.float32))
    return _RUNNER.run(s0, u_chunks, weights)  # (B, nsteps, S) fp32
